# revision 1
# baseline (speedup 1.0000x reference)
"""Trainium2 Bass/Tile kernel for nn_FB_FMM (sparse_attention).

Computation (per batch element b, with N = H*W = 4096 tokens, C=256, D=32):
  1. Self-attention:  sa_out = attn(conv(x,sa_wq), conv(x,sa_wk), conv(x,sa_wv))
     x' = sa_gamma * sa_out + x
  2. Masked cross-attention (FB_FMM):
     ff = mask * x'; fb = (1-mask) * x'
     sw_bg = attn(conv(ff,wq), conv(fb,wk), conv(fb,wv))
     out = x' + gamma * ff * (std(sw_bg)/std(ff))    [per-channel std, ddof=1]

Sharding: 8 cores = 2 batch groups x 4-way query-row sharding (1024 rows each).
Each core computes its row-chunk of both attention layers; K/V sides are
computed redundantly per core (cheap: D=32 / one C x C conv). Cross-core
communication inside the kernel:
  - AllGather of x' chunks within each 4-core batch group (layer-2 K/V need
    the full x'), split into two 512-row phases so the first overlaps the
    second half of the layer-1 attention loop.
  - AllReduce of per-channel [sum, sumsq] stats for the FMM std ratio.

Layouts: feature maps are channel-major (C on partitions). Scores are computed
transposed (S^T: keys j on partitions, queries i free; logits are small so exp
needs no max-subtraction pass). The AV matmul keeps V^T slices stationary
(weight reuse) with E^T moving, producing O in natural (c x i) layout; the
softmax denominator comes from one extra M=1 ones-matmul per tile, and the
reciprocal row is broadcast across partitions with a K=1 ones matmul. All
heavy matmuls run in float32r (1 cycle/row vs 4 for fp32; ~1e-4 relative
rounding). V-conv biases are folded out mathematically (sum_j A[i,j] = 1
makes the layer-1 V bias a constant shift folded into the residual; variance
is shift-invariant so the layer-2 V bias drops out of the FMM std).
"""

import numpy as np

P = 128
B, C, HH, WW = 2, 256, 64, 64
N = HH * WW            # 4096 tokens
D = 32                 # q/k channels
NCORES = 8
RSH = 4                # row shards per batch group
R = N // RSH           # 1024 query rows per core
NT = N // P            # 32 key tiles
IC = 512               # query i-chunk (one PSUM bank of fp32)
EPS = 1e-5
F32 = np.float32

_CACHE = {}


def _build_bass():
    """Build the Bass/Tile program (single SPMD NEFF for all 8 cores)."""
    import concourse.bass as bass
    from concourse import bacc, mybir, tile

    f32 = mybir.dt.float32
    f32r = mybir.dt.float32r
    bf16 = mybir.dt.bfloat16
    AX = mybir.AxisListType
    OP = mybir.AluOpType
    AF = mybir.ActivationFunctionType

    nc = bacc.Bacc(
        "TRN2", target_bir_lowering=False, debug=False, num_devices=NCORES
    )
    bf16d = mybir.dt.bfloat16

    # ---------------- I/O ----------------
    xf_d = nc.dram_tensor("xf", [C, N], bf16d, kind="ExternalInput")
    xc_d = nc.dram_tensor("xc", [C, R], f32r, kind="ExternalInput")
    mrow_d = nc.dram_tensor("mrow", [1, N], f32, kind="ExternalInput")
    mcrow_d = nc.dram_tensor("mcrow", [1, R], f32, kind="ExternalInput")
    wqT1_d = nc.dram_tensor("wqT1", [C, D], f32r, kind="ExternalInput")
    wkT1_d = nc.dram_tensor("wkT1", [C, D], bf16d, kind="ExternalInput")
    wvT1_d = nc.dram_tensor("wvT1", [C, C], bf16d, kind="ExternalInput")
    wqT2_d = nc.dram_tensor("wqT2", [C, D], f32r, kind="ExternalInput")
    wkT2_d = nc.dram_tensor("wkT2", [C, D], bf16d, kind="ExternalInput")
    wvT2_d = nc.dram_tensor("wvT2", [C, C], bf16d, kind="ExternalInput")
    # consts columns: 0 sa_gamma, 1 gamma, 2/3 sa_gamma*sa_bv halves,
    # 6 sa_bq, 7 sa_bk, 8 bq, 9 bk (cols 6-9 live on partitions 0..31)
    consts_d = nc.dram_tensor("consts", [P, 10], f32, kind="ExternalInput")
    out_d = nc.dram_tensor("outc", [C, R], f32, kind="ExternalOutput")

    groups = [[0, 1, 2, 3], [4, 5, 6, 7]]

    with tile.TileContext(nc) as tc:
        from contextlib import ExitStack

        ctx = ExitStack()
        with ctx:
            big = ctx.enter_context(tc.tile_pool(name="big", bufs=1))
            epool = ctx.enter_context(tc.tile_pool(name="epool", bufs=4))
            onpool = ctx.enter_context(tc.tile_pool(name="onpool", bufs=3))
            sqpool = ctx.enter_context(tc.tile_pool(name="sqpool", bufs=2))
            fbpool = ctx.enter_context(tc.tile_pool(name="fbpool", bufs=4))
            rcpool = ctx.enter_context(tc.tile_pool(name="rcpool", bufs=4))
            finpool = ctx.enter_context(tc.tile_pool(name="finpool", bufs=2))
            misc = ctx.enter_context(tc.tile_pool(name="misc", bufs=1))
            psA = ctx.enter_context(
                tc.tile_pool(name="psA", bufs=2, space="PSUM")
            )
            psS = ctx.enter_context(
                tc.tile_pool(name="psS", bufs=3, space="PSUM")
            )
            psO = ctx.enter_context(
                tc.tile_pool(name="psO", bufs=3, space="PSUM")
            )
            dram = ctx.enter_context(
                tc.tile_pool(name="dram", bufs=1, space="DRAM")
            )

            # ------------- persistent SBUF tiles -------------
            xc_sb = big.tile([P, 2, R], f32r, tag="xc", name="xc_sb")
            mask_sb = big.tile([P, N], f32, tag="mask", name="mask_sb")
            maskc_sb = big.tile([P, R], f32, tag="maskc", name="maskc_sb")
            xp_sb = big.tile([P, 2, R], f32, tag="xp", name="xp_sb")
            xp16_sb = big.tile([P, 2, R], bf16, tag="xp16", name="xp16_sb")
            ff_sb = big.tile([P, 2, R], f32r, tag="ff", name="ff_sb")
            wqT1_sb = big.tile([P, 2, D], f32r, tag="wqT1", name="wqT1_sb")
            wkT1_sb = big.tile([P, 2, D], bf16, tag="wkT1", name="wkT1_sb")
            wvT1_sb = big.tile([P, 2, C], bf16, tag="wvT1", name="wvT1_sb")
            wqT2_sb = big.tile([P, 2, D], f32r, tag="wqT2", name="wqT2_sb")
            wkT2_sb = big.tile([P, 2, D], bf16, tag="wkT2", name="wkT2_sb")
            wvT2_sb = big.tile([P, 2, C], bf16, tag="wvT2", name="wvT2_sb")
            consts_sb = big.tile([P, 10], f32, tag="consts", name="consts_sb")
            # ones column (f32r) for the denominator matmul; ones row (f32)
            # for the K=1 reciprocal-replication matmul
            onesc_sb = big.tile([P, 1], bf16, tag="onesc", name="onesc_sb")
            onesr_sb = big.tile([1, P], f32r, tag="onesr", name="onesr_sb")
            stats_sb = misc.tile([P, 8], f32, tag="stats", name="stats_sb")

            # input DMAs: small tensors first on the HWDGE queue; x-full
            # and the bf16 weight casts go through gpsimd (casting DMAs run
            # on a separate queue and overlap)
            xf_sb = big.tile([P, 2, N], bf16, tag="xbig", name="xf_sb")
            for k in range(2):
                for jc in range(4):
                    js = slice(jc * (N // 4), (jc + 1) * (N // 4))
                    nc.sync.dma_start(
                        out=xf_sb[:, k, js],
                        in_=xf_d[k * P : (k + 1) * P, js],
                    )
            nc.sync.dma_start(out=consts_sb[:], in_=consts_d[:])
            for k in range(2):
                cs = slice(k * P, (k + 1) * P)
                nc.sync.dma_start(out=wqT1_sb[:, k, :], in_=wqT1_d[cs, :])
                nc.sync.dma_start(out=wkT1_sb[:, k, :], in_=wkT1_d[cs, :])
                nc.sync.dma_start(out=wvT1_sb[:, k, :], in_=wvT1_d[cs, :])
                nc.sync.dma_start(out=xc_sb[:, k, :], in_=xc_d[cs, :])
                nc.sync.dma_start(out=wqT2_sb[:, k, :], in_=wqT2_d[cs, :])
                nc.sync.dma_start(out=wkT2_sb[:, k, :], in_=wkT2_d[cs, :])
                nc.sync.dma_start(out=wvT2_sb[:, k, :], in_=wvT2_d[cs, :])
            nc.sync.dma_start(
                out=mask_sb[:], in_=mrow_d[0, :].partition_broadcast(P)
            )
            nc.sync.dma_start(
                out=maskc_sb[:], in_=mcrow_d[0, :].partition_broadcast(P)
            )
            nc.vector.memset(onesc_sb[:], 1.0)
            nc.vector.memset(onesr_sb[:].bitcast(f32), 1.0)

            def conv_qk(wT_sb, bias_col, src_of, width, out_sb):
                """out (D x width) = wT.T @ src + bias.  src_of(k, js) gives
                the (128 x 512) input-channel tile."""
                for jc in range(width // IC):
                    js = slice(jc * IC, (jc + 1) * IC)
                    ps = psA.tile([D, IC], f32, tag="a", name="qk_ps")
                    nc.tensor.matmul(
                        ps[:], wT_sb[:, 0, :], src_of(0, js),
                        start=True, stop=False,
                    )
                    nc.tensor.matmul(
                        ps[:], wT_sb[:, 1, :], src_of(1, js),
                        start=False, stop=True,
                    )
                    nc.vector.tensor_scalar_add(
                        out_sb[:, js], ps[:],
                        consts_sb[0:D, bias_col : bias_col + 1],
                    )

            def conv_vT(wvT_sb, src_of, v_sb, t):
                """v_sb[:, t, :] = (src^T @ wvT) for key tile t (j on
                partitions, channels free)."""
                ts_ = slice(t * P, (t + 1) * P)
                ps = psA.tile([P, C], f32, tag="a", name="v_ps")
                nc.tensor.matmul(
                    ps[:], src_of(0, ts_), wvT_sb[:, 0, :],
                    start=True, stop=False,
                )
                nc.tensor.matmul(
                    ps[:], src_of(1, ts_), wvT_sb[:, 1, :],
                    start=False, stop=True,
                )
                nc.vector.tensor_copy(v_sb[:, t, :], ps[:])

            def attention(q_sb, k_sb, v_sb, epilogue):
                """Row-chunk attention.  Per i-chunk: S^T = K-tile^T Q
                (j on partitions), E = exp(S^T), then O(c,i) accumulates
                with V^T slices stationary and E moving; the denominator
                row comes from an M=1 ones matmul.  The S/exp stage is
                emitted two key-tiles ahead of AV/den so the in-order PE
                queue never stalls on the ACT exp.  epilogue(ich, accs,
                rrep) gets natural-layout unnormalized O accumulators and
                the partition-replicated reciprocal denominator (SBUF)."""
                LOOKAHEAD = 2
                for ich in range(R // IC):
                    is_ = slice(ich * IC, (ich + 1) * IC)
                    accs = [
                        psO.tile([P, IC], f32, tag="o", name="acc")
                        for _ in range(2)
                    ]
                    den = psA.tile([1, IC], f32, tag="a", name="den")
                    es = {}

                    def s_exp(t):
                        sps = psS.tile([P, IC], f32, tag="s", name="s_ps")
                        nc.tensor.matmul(
                            sps[:],
                            k_sb[:, t * P : (t + 1) * P],
                            q_sb[:, is_],
                            start=True, stop=True,
                        )
                        e_sb = epool.tile([P, IC], bf16, tag="e", name="e_sb")
                        nc.scalar.activation(e_sb[:], sps[:], AF.Exp)
                        es[t] = e_sb

                    for t in range(LOOKAHEAD):
                        s_exp(t)
                    for t in range(NT):
                        if t + LOOKAHEAD < NT:
                            s_exp(t + LOOKAHEAD)
                        e_sb = es.pop(t)
                        for ct in range(2):
                            nc.tensor.matmul(
                                accs[ct][:],
                                v_sb[:, t, ct * P : (ct + 1) * P],
                                e_sb[:],
                                start=(t == 0), stop=(t == NT - 1),
                            )
                        nc.tensor.matmul(
                            den[:], onesc_sb[:], e_sb[:],
                            start=(t == 0), stop=(t == NT - 1),
                        )
                    # reciprocal of the denominator row, replicated to all
                    # partitions via a K=1 ones matmul (f32r: single-pass)
                    rrow = rcpool.tile([1, IC], f32, tag="rc", name="rrow")
                    nc.vector.reciprocal(rrow[:], den[:])
                    rrow_r = rcpool.tile([1, IC], f32r, tag="rcr", name="rrow_r")
                    nc.vector.tensor_copy(rrow_r[:], rrow[:])
                    rrep_ps = psA.tile([P, IC], f32, tag="a", name="rrep_ps")
                    nc.tensor.matmul(
                        rrep_ps[:], onesr_sb[:], rrow_r[:],
                        start=True, stop=True,
                    )
                    rrep = onpool.tile([P, IC], f32, tag="rr", name="rrep")
                    nc.scalar.copy(rrep[:], rrep_ps[:])
                    epilogue(ich, accs, rrep)

            # ================= Layer 1: self-attention =================
            q1_sb = big.tile([D, R], bf16, tag="q", name="q1_sb")
            k1_sb = big.tile([D, N], bf16, tag="k", name="k1_sb")
            v1_sb = big.tile([P, NT, C], bf16, tag="v", name="v1_sb")

            conv_qk(wqT1_sb, 6, lambda k, js: xc_sb[:, k, js], R, q1_sb)
            conv_qk(wkT1_sb, 7, lambda k, js: xf_sb[:, k, js], N, k1_sb)
            for t in range(NT):
                conv_vT(wvT1_sb, lambda k, ts_: xf_sb[:, k, ts_], v1_sb, t)

            def epilogue1(ich, accs, rrep):
                io = slice(ich * IC, (ich + 1) * IC)
                for ct in range(2):
                    # x' = sa_gamma * (O/den) + sa_gamma*bv + x, fused as
                    # ((O * sa_gamma) * rrep), then ((t + sgb) + x)
                    nc.vector.scalar_tensor_tensor(
                        xp_sb[:, ct, io], accs[ct][:],
                        consts_sb[:, 0:1], rrep[:],
                        op0=OP.mult, op1=OP.mult,
                    )
                    nc.vector.scalar_tensor_tensor(
                        xp_sb[:, ct, io], xp_sb[:, ct, io],
                        consts_sb[:, 2 + ct : 3 + ct],
                        xc_sb[:, ct, io].bitcast(f32),
                        op0=OP.add, op1=OP.add,
                    )
                    nc.vector.tensor_copy(
                        xp16_sb[:, ct, io], xp_sb[:, ct, io]
                    )

            attention(q1_sb, k1_sb, v1_sb, epilogue1)

            # ====== AllGather x' within each batch group (2 phases) ======
            # Phase h gathers x' columns [h*512, (h+1)*512) of every rank;
            # phase 0 overlaps the second layer-1 attention i-chunk.
            xpf_sb = big.tile([P, 2, N], bf16, tag="xbig", name="xpf_sb")
            for h in range(2):
                hs = slice(h * IC, (h + 1) * IC)
                ag_in = dram.tile(
                    [C, IC], bf16, tag=f"ag_in{h}", name=f"ag_in{h}"
                )
                ag_out = dram.tile(
                    [RSH, C, IC], bf16, tag=f"ag_out{h}", name=f"ag_out{h}"
                )
                for ct in range(2):
                    nc.sync.dma_start(
                        out=ag_in[ct * P : (ct + 1) * P, :],
                        in_=xp16_sb[:, ct, hs],
                    )
                nc.gpsimd.collective_compute(
                    "AllGather",
                    OP.bypass,
                    replica_groups=groups,
                    ins=[ag_in[:].opt()],
                    outs=[ag_out[:].opt()],
                )
                for ct in range(2):
                    for r in range(RSH):
                        nc.sync.dma_start(
                            out=xpf_sb[
                                :, ct, r * R + h * IC : r * R + (h + 1) * IC
                            ],
                            in_=ag_out[r, ct * P : (ct + 1) * P, :],
                        )

            # ============== Layer 2: masked cross-attention ==============
            # feature_f chunk + its per-channel stats (cols 0-3 of stats_sb)
            ffsq = misc.tile([P, R], f32, tag="ffsq", name="ffsq")
            for ct in range(2):
                nc.vector.tensor_mul(
                    ff_sb[:, ct, :], maskc_sb[:], xp_sb[:, ct, :]
                )
                nc.vector.tensor_reduce(
                    stats_sb[:, ct : ct + 1], ff_sb[:, ct, :].bitcast(f32),
                    axis=AX.X, op=OP.add,
                )
                nc.vector.tensor_mul(
                    ffsq[:],
                    ff_sb[:, ct, :].bitcast(f32),
                    ff_sb[:, ct, :].bitcast(f32),
                )
                nc.vector.tensor_reduce(
                    stats_sb[:, 2 + ct : 3 + ct], ffsq[:],
                    axis=AX.X, op=OP.add,
                )

            q2_sb = big.tile([D, R], bf16, tag="q", name="q2_sb")
            conv_qk(wqT2_sb, 8, lambda k, js: ff_sb[:, k, js], R, q2_sb)

            # feature_b tiles on the fly -> K2 and V2^T convs.  Chunk order
            # interleaves gather phases: even chunks only need AG phase 0.
            k2_sb = big.tile([D, N], bf16, tag="k", name="k2_sb")
            v2_sb = big.tile([P, NT, C], bf16, tag="v", name="v2_sb")
            for jc in (0, 2, 4, 6, 1, 3, 5, 7):
                js = slice(jc * IC, (jc + 1) * IC)
                fb0 = fbpool.tile([P, IC], bf16, tag="fb", name="fb0")
                fb1 = fbpool.tile([P, IC], bf16, tag="fb", name="fb1")
                # fb = (1-mask)*x' = x' - mask*x'
                nc.vector.tensor_mul(
                    fb0[:], mask_sb[:, js], xpf_sb[:, 0, js]
                )
                nc.vector.tensor_sub(
                    fb0[:], xpf_sb[:, 0, js], fb0[:]
                )
                nc.vector.tensor_mul(
                    fb1[:], mask_sb[:, js], xpf_sb[:, 1, js]
                )
                nc.vector.tensor_sub(
                    fb1[:], xpf_sb[:, 1, js], fb1[:]
                )
                ps = psA.tile([D, IC], f32, tag="a", name="k2_ps")
                nc.tensor.matmul(
                    ps[:], wkT2_sb[:, 0, :], fb0[:], start=True, stop=False
                )
                nc.tensor.matmul(
                    ps[:], wkT2_sb[:, 1, :], fb1[:], start=False, stop=True
                )
                nc.vector.tensor_scalar_add(
                    k2_sb[:, js], ps[:], consts_sb[0:D, 9:10]
                )
                for tsub in range(IC // P):
                    t = jc * (IC // P) + tsub
                    ts_ = slice(tsub * P, (tsub + 1) * P)
                    psv = psA.tile([P, C], f32, tag="a", name="v2_ps")
                    nc.tensor.matmul(
                        psv[:], fb0[:, ts_], wvT2_sb[:, 0, :],
                        start=True, stop=False,
                    )
                    nc.tensor.matmul(
                        psv[:], fb1[:, ts_], wvT2_sb[:, 1, :],
                        start=False, stop=True,
                    )
                    nc.vector.tensor_copy(v2_sb[:, t, :], psv[:])

            def epilogue2(ich, accs, rrep):
                # normalized sw_bg chunk in natural layout; accumulate
                # per-channel sum/sumsq into stats_sb cols 4-7 via VE
                for ct in range(2):
                    onb = onpool.tile([P, IC], f32, tag="on", name="on2")
                    s1 = rcpool.tile([P, 1], f32, tag="s1", name="s1")
                    nc.vector.scalar_tensor_tensor(
                        onb[:], accs[ct][:], 1.0, rrep[:],
                        op0=OP.mult, op1=OP.mult, accum_out=s1[:],
                    )
                    sqb = sqpool.tile([P, IC], f32, tag="sq", name="sq2")
                    s2 = rcpool.tile([P, 1], f32, tag="s2", name="s2")
                    nc.vector.scalar_tensor_tensor(
                        sqb[:], onb[:], 1.0, onb[:],
                        op0=OP.mult, op1=OP.mult, accum_out=s2[:],
                    )
                    if ich == 0:
                        nc.vector.tensor_copy(
                            stats_sb[:, 4 + ct : 5 + ct], s1[:]
                        )
                        nc.vector.tensor_copy(
                            stats_sb[:, 6 + ct : 7 + ct], s2[:]
                        )
                    else:
                        nc.vector.tensor_add(
                            stats_sb[:, 4 + ct : 5 + ct],
                            stats_sb[:, 4 + ct : 5 + ct], s1[:],
                        )
                        nc.vector.tensor_add(
                            stats_sb[:, 6 + ct : 7 + ct],
                            stats_sb[:, 6 + ct : 7 + ct], s2[:],
                        )

            attention(q2_sb, k2_sb, v2_sb, epilogue2)

            # ================== stats AllReduce + FMM ==================
            ar_in = dram.tile([P, 8], f32, tag="ar_in", name="ar_in")
            ar_out = dram.tile([P, 8], f32, tag="ar_out", name="ar_out")
            nc.sync.dma_start(out=ar_in[:], in_=stats_sb[:])
            nc.gpsimd.collective_compute(
                "AllReduce",
                OP.add,
                replica_groups=groups,
                ins=[ar_in[:].opt()],
                outs=[ar_out[:].opt()],
            )
            rst = misc.tile([P, 8], f32, tag="rst", name="rst")
            nc.sync.dma_start(out=rst[:], in_=ar_out[:])

            # var = (S2 - S1^2/N)/(N-1) + EPS (both channel-halves at once)
            varf = misc.tile([P, 2], f32, tag="varf", name="varf")
            varg = misc.tile([P, 2], f32, tag="varg", name="varg")
            ratio = misc.tile([P, 2], f32, tag="ratio", name="ratio")
            for var, s1s, s2s in ((varf, 0, 2), (varg, 4, 6)):
                nc.vector.tensor_mul(
                    var[:], rst[:, s1s : s1s + 2], rst[:, s1s : s1s + 2]
                )
                nc.vector.tensor_scalar(
                    var[:], var[:], -1.0 / N, None, op0=OP.mult
                )
                nc.vector.tensor_add(var[:], var[:], rst[:, s2s : s2s + 2])
                nc.vector.tensor_scalar(
                    var[:], var[:], 1.0 / (N - 1), EPS, op0=OP.mult, op1=OP.add
                )
            nc.vector.reciprocal(varf[:], varf[:])
            nc.vector.tensor_mul(varg[:], varg[:], varf[:])
            nc.scalar.activation(ratio[:], varg[:], AF.Sqrt)
            # fold in gamma
            nc.vector.tensor_scalar_mul(ratio[:], ratio[:], consts_sb[:, 1:2])

            # out = x' + (gamma * std_bg/std_f) * ff
            for ct in range(2):
                fin = finpool.tile([P, R], f32, tag="fin", name="fin")
                nc.vector.scalar_tensor_tensor(
                    fin[:], ff_sb[:, ct, :].bitcast(f32),
                    ratio[:, ct : ct + 1], xp_sb[:, ct, :],
                    op0=OP.mult, op1=OP.add,
                )
                nc.sync.dma_start(
                    out=out_d[ct * P : (ct + 1) * P, :], in_=fin[:]
                )

    nc.compile()
    return nc


def _prep_inputs(x, mask, sa_wq, sa_bq, sa_wk, sa_bk, sa_wv, sa_bv, sa_gamma,
                 wq, bq, wk, bk, wv, bv, gamma):
    """Build the per-core input maps (host-side sharding + weight layout)."""
    x = np.ascontiguousarray(x, dtype=F32)
    mask = np.ascontiguousarray(mask, dtype=F32)

    import ml_dtypes

    BF16 = ml_dtypes.bfloat16
    wqT1 = np.ascontiguousarray(sa_wq.T, dtype=F32)
    wkT1 = np.ascontiguousarray(sa_wk.T.astype(BF16))
    wvT1 = np.ascontiguousarray(sa_wv.T.astype(BF16))
    wqT2 = np.ascontiguousarray(wq.T, dtype=F32)
    wkT2 = np.ascontiguousarray(wk.T.astype(BF16))
    wvT2 = np.ascontiguousarray(wv.T.astype(BF16))

    consts = np.zeros((P, 10), dtype=F32)
    consts[:, 0] = sa_gamma[0]
    consts[:, 1] = gamma[0]
    sgb = (sa_gamma[0] * sa_bv).astype(F32)
    consts[:, 2] = sgb[0:P]
    consts[:, 3] = sgb[P:C]
    consts[0:D, 6] = sa_bq
    consts[0:D, 7] = sa_bk
    consts[0:D, 8] = bq
    consts[0:D, 9] = bk

    in_maps = []
    for g in range(NCORES):
        b, r = g // RSH, g % RSH
        xb = np.ascontiguousarray(x[b].reshape(C, N))
        mb = np.ascontiguousarray(mask[b].reshape(1, N))
        in_maps.append({
            "xf": np.ascontiguousarray(xb.astype(BF16)),
            "xc": np.ascontiguousarray(xb[:, r * R : (r + 1) * R]),
            "mrow": mb,
            "mcrow": np.ascontiguousarray(mb[:, r * R : (r + 1) * R]),
            "wqT1": wqT1, "wkT1": wkT1, "wvT1": wvT1,
            "wqT2": wqT2, "wkT2": wkT2, "wvT2": wvT2,
            "consts": consts,
        })
    return in_maps


def kernel(**inputs):
    from concourse import bass_utils

    if "nc" not in _CACHE:
        _CACHE["nc"] = _build_bass()
    nc = _CACHE["nc"]

    in_maps = _prep_inputs(**inputs)
    res = bass_utils.run_bass_kernel_spmd(
        nc, in_maps, core_ids=list(range(NCORES))
    )
    _CACHE["last_results"] = res

    out = np.empty((B, C, N), dtype=F32)
    for g in range(NCORES):
        b, r = g // RSH, g % RSH
        out[b, :, r * R : (r + 1) * R] = res.results[g]["outc"]
    return out.reshape(B, C, HH, WW)



# revision 6
# speedup vs baseline: 1.1962x; 1.1962x over previous
"""Trainium2 Bass/Tile kernel for nn_FB_FMM (sparse_attention).

Computation (per batch element b, with N = H*W = 4096 tokens, C=256, D=32):
  1. Self-attention:  sa_out = attn(conv(x,sa_wq), conv(x,sa_wk), conv(x,sa_wv))
     x' = sa_gamma * sa_out + x
  2. Masked cross-attention (FB_FMM):
     ff = mask * x'; fb = (1-mask) * x'
     sw_bg = attn(conv(ff,wq), conv(fb,wk), conv(fb,wv))
     out = x' + gamma * ff * (std(sw_bg)/std(ff))    [per-channel std, ddof=1]

Sharding: 8 cores = 2 batch groups x 4-way query-row sharding (1024 rows each).
Each core computes its row-chunk of both attention layers; K/V sides are
computed redundantly per core. Cross-core communication inside the kernel:
  - AllGather of fb = (1-mask)x' chunks within each 4-core batch group
    (layer-2 K/V convs need full fb; gathering fb instead of x' removes all
    post-gather mask arithmetic), split into two 512-row phases so phase 0
    overlaps the second layer-1 attention i-chunk.
  - AllGather (+ local 3-add sum) of per-channel [sum, sumsq] stats for the
    FMM std ratio -- cheaper than a CC AllReduce for 4KB.

Layouts: feature maps are channel-major (C on partitions). Scores are computed
transposed (S^T: keys j on partitions, queries i free; logits are small so exp
needs no max-subtraction pass). The AV matmul keeps V^T slices stationary
with E^T moving, producing O in natural (c x i) layout; the softmax
denominator comes from one extra M=1 ones-matmul per tile, and the
reciprocal row (reciprocal_approx_fast, ~18 bits) is broadcast across
partitions with a K=1 ones matmul. Layer-1 K/V convs are emitted interleaved
with the i-chunk-0 attention tiles so the PE starts as soon as the first
input chunk lands instead of after the full 2MB x DMA.
"""

import numpy as np

P = 128
B, C, HH, WW = 2, 256, 64, 64
N = HH * WW            # 4096 tokens
D = 32                 # q/k channels
NCORES = 8
RSH = 4                # row shards per batch group
R = N // RSH           # 1024 query rows per core
NT = N // P            # 32 key tiles
IC = 512               # query i-chunk (one PSUM bank of fp32)
EPS = 1e-5
F32 = np.float32

_CACHE = {}


def _build_bass():
    """Build the Bass/Tile program (single SPMD NEFF for all 8 cores)."""
    import concourse.bass as bass
    from concourse import bacc, mybir, tile

    f32 = mybir.dt.float32
    f32r = mybir.dt.float32r
    bf16 = mybir.dt.bfloat16
    AX = mybir.AxisListType
    OP = mybir.AluOpType
    AF = mybir.ActivationFunctionType

    nc = bacc.Bacc(
        "TRN2", target_bir_lowering=False, debug=False, num_devices=NCORES
    )
    bf16d = mybir.dt.bfloat16

    # ---------------- I/O ----------------
    xf_d = nc.dram_tensor("xf", [C, N], bf16d, kind="ExternalInput")
    xc_d = nc.dram_tensor("xc", [C, R], f32r, kind="ExternalInput")
    mcrow_d = nc.dram_tensor("mcrow", [1, R], f32, kind="ExternalInput")
    wqT1_d = nc.dram_tensor("wqT1", [C, D], f32r, kind="ExternalInput")
    wkT1_d = nc.dram_tensor("wkT1", [C, D], bf16d, kind="ExternalInput")
    wvT1_d = nc.dram_tensor("wvT1", [C, C], bf16d, kind="ExternalInput")
    wqT2_d = nc.dram_tensor("wqT2", [C, D], f32r, kind="ExternalInput")
    wkT2_d = nc.dram_tensor("wkT2", [C, D], bf16d, kind="ExternalInput")
    wvT2_d = nc.dram_tensor("wvT2", [C, C], bf16d, kind="ExternalInput")
    # consts columns: 0 sa_gamma, 1 gamma, 2/3 sa_gamma*sa_bv halves,
    # 6 sa_bq, 7 sa_bk, 8 bq, 9 bk (cols 6-9 live on partitions 0..31)
    consts_d = nc.dram_tensor("consts", [P, 10], f32, kind="ExternalInput")
    out_d = nc.dram_tensor("outc", [C, R], f32, kind="ExternalOutput")

    groups = [[0, 1, 2, 3], [4, 5, 6, 7]]

    with tile.TileContext(nc) as tc:
        from contextlib import ExitStack

        ctx = ExitStack()
        with ctx:
            big = ctx.enter_context(tc.tile_pool(name="big", bufs=1))
            epool = ctx.enter_context(tc.tile_pool(name="epool", bufs=4))
            onpool = ctx.enter_context(tc.tile_pool(name="onpool", bufs=3))
            sqpool = ctx.enter_context(tc.tile_pool(name="sqpool", bufs=2))
            rcpool = ctx.enter_context(tc.tile_pool(name="rcpool", bufs=2))
            finpool = ctx.enter_context(tc.tile_pool(name="finpool", bufs=2))
            misc = ctx.enter_context(tc.tile_pool(name="misc", bufs=1))
            # PSUM: accs (3 rotating) + S^T (2) + convs (2) + den/rrep (1)
            psO = ctx.enter_context(
                tc.tile_pool(name="psO", bufs=3, space="PSUM")
            )
            psS = ctx.enter_context(
                tc.tile_pool(name="psS", bufs=2, space="PSUM")
            )
            psC = ctx.enter_context(
                tc.tile_pool(name="psC", bufs=2, space="PSUM")
            )
            psD = ctx.enter_context(
                tc.tile_pool(name="psD", bufs=1, space="PSUM")
            )
            dram = ctx.enter_context(
                tc.tile_pool(name="dram", bufs=1, space="DRAM")
            )

            # ------------- persistent SBUF tiles -------------
            xc_sb = big.tile([P, 2, R], f32r, tag="xc", name="xc_sb")
            maskc_sb = big.tile([P, R], f32, tag="maskc", name="maskc_sb")
            xp_sb = big.tile([P, 2, R], f32, tag="xp", name="xp_sb")
            ff_sb = big.tile([P, 2, R], f32r, tag="ff", name="ff_sb")
            fb16_sb = big.tile([P, 2, R], bf16, tag="fb16", name="fb16_sb")
            wqT1_sb = big.tile([P, 2, D], f32r, tag="wqT1", name="wqT1_sb")
            wkT1_sb = big.tile([P, 2, D], bf16, tag="wkT1", name="wkT1_sb")
            wvT1_sb = big.tile([P, 2, C], bf16, tag="wvT1", name="wvT1_sb")
            wqT2_sb = big.tile([P, 2, D], f32r, tag="wqT2", name="wqT2_sb")
            wkT2_sb = big.tile([P, 2, D], bf16, tag="wkT2", name="wkT2_sb")
            wvT2_sb = big.tile([P, 2, C], bf16, tag="wvT2", name="wvT2_sb")
            consts_sb = big.tile([P, 10], f32, tag="consts", name="consts_sb")
            # ones column (bf16) for the denominator matmul; ones row (f32r)
            # for the K=1 reciprocal-replication matmul
            onesc_sb = big.tile([P, 1], bf16, tag="onesc", name="onesc_sb")
            onesr_sb = big.tile([1, P], f32r, tag="onesr", name="onesr_sb")
            stats_sb = misc.tile([P, 8], f32, tag="stats", name="stats_sb")
            # per-(ct, ich) stat accumulator columns: 4 tensors x [ct, ich]
            sacc_sb = misc.tile([P, 16], f32, tag="sacc", name="sacc_sb")

            # input DMAs: small tensors first on the HWDGE queue, then xc,
            # then x-full in 8 column chunks so layer-1 convs can start as
            # soon as the first chunk lands.
            nc.sync.dma_start(out=consts_sb[:], in_=consts_d[:])
            for k in range(2):
                cs = slice(k * P, (k + 1) * P)
                nc.sync.dma_start(out=wqT1_sb[:, k, :], in_=wqT1_d[cs, :])
                nc.sync.dma_start(out=wkT1_sb[:, k, :], in_=wkT1_d[cs, :])
                nc.sync.dma_start(out=wvT1_sb[:, k, :], in_=wvT1_d[cs, :])
                nc.sync.dma_start(out=wqT2_sb[:, k, :], in_=wqT2_d[cs, :])
                nc.sync.dma_start(out=wkT2_sb[:, k, :], in_=wkT2_d[cs, :])
                nc.sync.dma_start(out=wvT2_sb[:, k, :], in_=wvT2_d[cs, :])
            for k in range(2):
                nc.sync.dma_start(
                    out=xc_sb[:, k, :], in_=xc_d[k * P : (k + 1) * P, :]
                )
            nc.sync.dma_start(
                out=maskc_sb[:], in_=mcrow_d[0, :].partition_broadcast(P)
            )
            xf_sb = big.tile([P, 2, N], bf16, tag="xbig", name="xf_sb")
            NCH = 8                       # xf DMA chunks
            CW = N // NCH                 # 512 columns per chunk
            for jc in range(NCH):
                js = slice(jc * CW, (jc + 1) * CW)
                for k in range(2):
                    nc.sync.dma_start(
                        out=xf_sb[:, k, js],
                        in_=xf_d[k * P : (k + 1) * P, js],
                    )
            nc.vector.memset(onesc_sb[:], 1.0)
            nc.vector.memset(onesr_sb[:].bitcast(f32), 1.0)

            def conv_qk(wT_sb, bias_col, src_of, cols, out_sb):
                """out[:, cols] (D x 512-chunks) = wT.T @ src + bias."""
                for jc in range(cols.start // IC, cols.stop // IC):
                    js = slice(jc * IC, (jc + 1) * IC)
                    ps = psC.tile([D, IC], f32, tag="c", name="qk_ps")
                    nc.tensor.matmul(
                        ps[:], wT_sb[:, 0, :], src_of(0, js),
                        start=True, stop=False,
                    )
                    nc.tensor.matmul(
                        ps[:], wT_sb[:, 1, :], src_of(1, js),
                        start=False, stop=True,
                    )
                    nc.vector.tensor_scalar_add(
                        out_sb[:, js], ps[:],
                        consts_sb[0:D, bias_col : bias_col + 1],
                    )

            def conv_vT(wvT_sb, src_of, v_sb, t):
                """v_sb[:, t, :] = (src^T @ wvT) for key tile t (j on
                partitions, channels free)."""
                ts_ = slice(t * P, (t + 1) * P)
                ps = psC.tile([P, C], f32, tag="c", name="v_ps")
                nc.tensor.matmul(
                    ps[:], src_of(0, ts_), wvT_sb[:, 0, :],
                    start=True, stop=False,
                )
                nc.tensor.matmul(
                    ps[:], src_of(1, ts_), wvT_sb[:, 1, :],
                    start=False, stop=True,
                )
                nc.vector.tensor_copy(v_sb[:, t, :], ps[:])

            class AttnChunk:
                """One query i-chunk of row-sharded attention, with tile
                emission split into arbitrary sub-sequences so conv work can
                be interleaved.  S^T = K-tile^T Q (j on partitions),
                E = exp(S^T), O accumulates with V^T stationary and E moving;
                denominator via an M=1 ones-matmul per tile."""

                def __init__(self, q_sb, k_sb, v_sb, ich, order):
                    self.q_sb, self.k_sb, self.v_sb = q_sb, k_sb, v_sb
                    self.is_ = slice(ich * IC, (ich + 1) * IC)
                    self.order = order
                    self.pos = 0          # next order index to AV
                    self.ahead = 0        # next order index to S/exp
                    self.es = {}
                    self.accs = [
                        psO.tile([P, IC], f32, tag="o", name="acc")
                        for _ in range(2)
                    ]
                    self.den = psD.tile([1, IC], f32, tag="d", name="den")

                def _s_exp(self):
                    t = self.order[self.ahead]
                    self.ahead += 1
                    sps = psS.tile([P, IC], f32, tag="s", name="s_ps")
                    nc.tensor.matmul(
                        sps[:],
                        self.k_sb[:, t * P : (t + 1) * P],
                        self.q_sb[:, self.is_],
                        start=True, stop=True,
                    )
                    e_sb = epool.tile([P, IC], bf16, tag="e", name="e_sb")
                    nc.scalar.activation(e_sb[:], sps[:], AF.Exp)
                    self.es[t] = e_sb

                def emit(self, n):
                    """Emit the next n tiles' worth of S/exp/AV/den."""
                    LOOKAHEAD = 1
                    for _ in range(n):
                        while (
                            self.ahead < len(self.order)
                            and self.ahead <= self.pos + LOOKAHEAD
                        ):
                            self._s_exp()
                        t = self.order[self.pos]
                        first = self.pos == 0
                        last = self.pos == len(self.order) - 1
                        self.pos += 1
                        e_sb = self.es.pop(t)
                        for ct in range(2):
                            nc.tensor.matmul(
                                self.accs[ct][:],
                                self.v_sb[:, t, ct * P : (ct + 1) * P],
                                e_sb[:],
                                start=first, stop=last,
                            )
                        nc.tensor.matmul(
                            self.den[:], onesc_sb[:], e_sb[:],
                            start=first, stop=last,
                        )

                def rrep(self):
                    """Reciprocal of the denominator row, replicated to all
                    partitions via a K=1 ones matmul (f32r single-pass).
                    reciprocal_approx_fast (~18 bits) replaces the 5x slower
                    exact DVE reciprocal; the f32r copy rounds for the PE."""
                    rrow = rcpool.tile([1, IC], f32, tag="rc0", name="rw")
                    nc.vector.reciprocal_approx_fast(
                        out=rrow[:], in_=self.den[:]
                    )
                    rrow_r = rcpool.tile([1, IC], f32r, tag="rc", name="rr")
                    nc.vector.tensor_copy(rrow_r[:], rrow[:])
                    rrep_ps = psD.tile([P, IC], f32, tag="d", name="rrep_ps")
                    nc.tensor.matmul(
                        rrep_ps[:], onesr_sb[:], rrow_r[:],
                        start=True, stop=True,
                    )
                    rr = onpool.tile([P, IC], f32, tag="rr", name="rrep")
                    nc.scalar.copy(rr[:], rrep_ps[:])
                    return rr

            # ================= Layer 1: self-attention =================
            q1_sb = big.tile([D, R], bf16, tag="q", name="q1_sb")
            k1_sb = big.tile([D, N], bf16, tag="k", name="k1_sb")
            v1_sb = big.tile([P, NT, C], bf16, tag="v", name="v1_sb")

            conv_qk(wqT1_sb, 6, lambda k, js: xc_sb[:, k, js], slice(0, R),
                    q1_sb)

            def l1_conv_chunk(c):
                js = slice(c * CW, (c + 1) * CW)
                conv_qk(wkT1_sb, 7, lambda k, js_: xf_sb[:, k, js_], js,
                        k1_sb)
                for t in range(4 * c, 4 * c + 4):
                    conv_vT(wvT1_sb, lambda k, ts_: xf_sb[:, k, ts_],
                            v1_sb, t)

            # i-chunk 0 attention trails the conv chunks by one chunk
            at0 = AttnChunk(q1_sb, k1_sb, v1_sb, 0, list(range(NT)))
            l1_conv_chunk(0)
            for c in range(1, NCH):
                l1_conv_chunk(c)
                at0.emit(4)
            at0.emit(4)

            ag_ins, ag_outs = [], []
            for h in range(2):
                ag_ins.append(dram.tile(
                    [C, IC], bf16, tag=f"ag_in{h}", name=f"ag_in{h}"
                ))
                ag_outs.append(dram.tile(
                    [RSH, C, IC], bf16, tag=f"ag_out{h}", name=f"ag_out{h}"
                ))

            def epilogue1(ich, at):
                """x' = sa_gamma*(O/den) + sa_gamma*bv + x, then ff = m*x'
                (f32r, with Σff accumulated), fb16 = x' - ff for the gather,
                and Σff² -- stats land in sacc_sb columns."""
                rr = at.rrep()
                io = slice(ich * IC, (ich + 1) * IC)
                for ct in range(2):
                    nc.vector.scalar_tensor_tensor(
                        xp_sb[:, ct, io], at.accs[ct][:],
                        consts_sb[:, 0:1], rr[:],
                        op0=OP.mult, op1=OP.mult,
                    )
                    nc.vector.scalar_tensor_tensor(
                        xp_sb[:, ct, io], xp_sb[:, ct, io],
                        consts_sb[:, 2 + ct : 3 + ct],
                        xc_sb[:, ct, io].bitcast(f32),
                        op0=OP.add, op1=OP.add,
                    )
                    nc.vector.scalar_tensor_tensor(
                        ff_sb[:, ct, io], xp_sb[:, ct, io],
                        1.0, maskc_sb[:, io],
                        op0=OP.mult, op1=OP.mult,
                        accum_out=sacc_sb[:, 2 * ct + ich : 2 * ct + ich + 1],
                    )
                    nc.vector.tensor_sub(
                        fb16_sb[:, ct, io], xp_sb[:, ct, io],
                        ff_sb[:, ct, io].bitcast(f32),
                    )
                # AllGather this i-chunk's fb columns ASAP
                for ct in range(2):
                    nc.sync.dma_start(
                        out=ag_ins[ich][ct * P : (ct + 1) * P, :],
                        in_=fb16_sb[:, ct, io],
                    )
                nc.gpsimd.collective_compute(
                    "AllGather",
                    OP.bypass,
                    replica_groups=groups,
                    ins=[ag_ins[ich][:].opt()],
                    outs=[ag_outs[ich][:].opt()],
                )
                # Σff² (scratch output, accumulator is the point)
                for ct in range(2):
                    sq = sqpool.tile([P, IC], f32, tag="sq", name="ffsq")
                    nc.vector.scalar_tensor_tensor(
                        sq[:], ff_sb[:, ct, io].bitcast(f32),
                        1.0, ff_sb[:, ct, io].bitcast(f32),
                        op0=OP.mult, op1=OP.mult,
                        accum_out=sacc_sb[:, 4 + 2 * ct + ich
                                          : 5 + 2 * ct + ich],
                    )

            epilogue1(0, at0)
            del at0

            at1 = AttnChunk(q1_sb, k1_sb, v1_sb, 1, list(range(NT)))
            at1.emit(NT)
            epilogue1(1, at1)
            del at1

            # assemble ff stats (cols 0-3 of stats_sb)
            for c in range(4):
                nc.vector.tensor_add(
                    stats_sb[:, c : c + 1],
                    sacc_sb[:, 2 * c : 2 * c + 1],
                    sacc_sb[:, 2 * c + 1 : 2 * c + 2],
                )

            # ============== Layer 2: masked cross-attention ==============
            q2_sb = big.tile([D, R], bf16, tag="q", name="q2_sb")
            conv_qk(wqT2_sb, 8, lambda k, js: ff_sb[:, k, js], slice(0, R),
                    q2_sb)

            # gathered fb landing buffer; phase h delivers columns
            # [r*R + h*IC, r*R + (h+1)*IC) for every rank r
            fbf_sb = big.tile([P, 2, N], bf16, tag="xbig", name="fbf_sb")
            for h in range(2):
                for ct in range(2):
                    for r in range(RSH):
                        nc.sync.dma_start(
                            out=fbf_sb[
                                :, ct, r * R + h * IC : r * R + (h + 1) * IC
                            ],
                            in_=ag_outs[h][r, ct * P : (ct + 1) * P, :],
                        )

            k2_sb = big.tile([D, N], bf16, tag="k", name="k2_sb")
            v2_sb = big.tile([P, NT, C], bf16, tag="v", name="v2_sb")

            def l2_conv_block(r, h):
                js = slice(r * R + h * IC, r * R + (h + 1) * IC)
                conv_qk(wkT2_sb, 9, lambda k, js_: fbf_sb[:, k, js_], js,
                        k2_sb)
                t0 = (r * R + h * IC) // P
                for t in range(t0, t0 + 4):
                    conv_vT(wvT2_sb, lambda k, ts_: fbf_sb[:, k, ts_],
                            v2_sb, t)

            ph_tiles = [
                [t for r in range(RSH)
                 for t in range((r * R + h * IC) // P,
                                (r * R + h * IC) // P + 4)]
                for h in range(2)
            ]

            for r in range(RSH):
                l2_conv_block(r, 0)
            bt0 = AttnChunk(q2_sb, k2_sb, v2_sb, 0,
                            ph_tiles[0] + ph_tiles[1])
            bt0.emit(10)
            for r in range(RSH):
                l2_conv_block(r, 1)
            bt0.emit(NT - 10)

            def epilogue2(ich, at):
                """normalized sw_bg chunk; accumulate per-channel sum/sumsq
                into sacc_sb cols 8-15 via the DVE accumulator."""
                rr = at.rrep()
                for ct in range(2):
                    onb = onpool.tile([P, IC], f32, tag="rr", name="on2")
                    nc.vector.scalar_tensor_tensor(
                        onb[:], at.accs[ct][:], 1.0, rr[:],
                        op0=OP.mult, op1=OP.mult,
                        accum_out=sacc_sb[:, 8 + 2 * ct + ich
                                          : 9 + 2 * ct + ich],
                    )
                    sqb = sqpool.tile([P, IC], f32, tag="sq", name="sq2")
                    nc.vector.scalar_tensor_tensor(
                        sqb[:], onb[:], 1.0, onb[:],
                        op0=OP.mult, op1=OP.mult,
                        accum_out=sacc_sb[:, 12 + 2 * ct + ich
                                          : 13 + 2 * ct + ich],
                    )

            epilogue2(0, bt0)
            del bt0
            bt1 = AttnChunk(q2_sb, k2_sb, v2_sb, 1, list(range(NT)))
            bt1.emit(NT)
            epilogue2(1, bt1)
            del bt1

            # assemble sw_bg stats (cols 4-7)
            for c in range(4):
                nc.vector.tensor_add(
                    stats_sb[:, 4 + c : 5 + c],
                    sacc_sb[:, 8 + 2 * c : 9 + 2 * c],
                    sacc_sb[:, 9 + 2 * c : 10 + 2 * c],
                )

            # ============ stats AllGather + local sum + FMM ============
            sg_in = dram.tile([P, 8], f32, tag="sg_in", name="sg_in")
            sg_out = dram.tile([RSH, P, 8], f32, tag="sg_out", name="sg_out")
            nc.sync.dma_start(out=sg_in[:], in_=stats_sb[:])
            nc.gpsimd.collective_compute(
                "AllGather",
                OP.bypass,
                replica_groups=groups,
                ins=[sg_in[:].opt()],
                outs=[sg_out[:].opt()],
            )
            rst4 = misc.tile([P, RSH, 8], f32, tag="rst4", name="rst4")
            for r in range(RSH):
                nc.sync.dma_start(out=rst4[:, r, :], in_=sg_out[r, :, :])
            rst = misc.tile([P, 8], f32, tag="rst", name="rst")
            nc.vector.tensor_add(rst[:], rst4[:, 0, :], rst4[:, 1, :])
            rstb = misc.tile([P, 8], f32, tag="rstb", name="rstb")
            nc.vector.tensor_add(rstb[:], rst4[:, 2, :], rst4[:, 3, :])
            nc.vector.tensor_add(rst[:], rst[:], rstb[:])

            # var = (S2 - S1^2/N)/(N-1) + EPS (both channel-halves at once)
            varf = misc.tile([P, 2], f32, tag="varf", name="varf")
            varg = misc.tile([P, 2], f32, tag="varg", name="varg")
            ratio = misc.tile([P, 2], f32, tag="ratio", name="ratio")
            for var, s1s, s2s in ((varf, 0, 2), (varg, 4, 6)):
                nc.vector.tensor_mul(
                    var[:], rst[:, s1s : s1s + 2], rst[:, s1s : s1s + 2]
                )
                nc.vector.tensor_scalar(
                    var[:], var[:], -1.0 / N, None, op0=OP.mult
                )
                nc.vector.tensor_add(var[:], var[:], rst[:, s2s : s2s + 2])
                nc.vector.tensor_scalar(
                    var[:], var[:], 1.0 / (N - 1), EPS, op0=OP.mult, op1=OP.add
                )
            nc.vector.reciprocal(varf[:], varf[:])
            nc.vector.tensor_mul(varg[:], varg[:], varf[:])
            nc.scalar.activation(ratio[:], varg[:], AF.Sqrt)
            # fold in gamma
            nc.vector.tensor_scalar_mul(ratio[:], ratio[:], consts_sb[:, 1:2])

            # out = x' + (gamma * std_bg/std_f) * ff, in 4 chunks so the
            # store DMAs start as soon as the first chunk is ready
            for ct in range(2):
                for ih in range(2):
                    io = slice(ih * IC, (ih + 1) * IC)
                    fin = finpool.tile([P, IC], f32, tag="fin", name="fin")
                    nc.vector.scalar_tensor_tensor(
                        fin[:], ff_sb[:, ct, io].bitcast(f32),
                        ratio[:, ct : ct + 1], xp_sb[:, ct, io],
                        op0=OP.mult, op1=OP.add,
                    )
                    nc.sync.dma_start(
                        out=out_d[ct * P : (ct + 1) * P, io], in_=fin[:]
                    )

    nc.compile()
    return nc


def _prep_inputs(x, mask, sa_wq, sa_bq, sa_wk, sa_bk, sa_wv, sa_bv, sa_gamma,
                 wq, bq, wk, bk, wv, bv, gamma):
    """Build the per-core input maps (host-side sharding + weight layout)."""
    x = np.ascontiguousarray(x, dtype=F32)
    mask = np.ascontiguousarray(mask, dtype=F32)

    import ml_dtypes

    BF16 = ml_dtypes.bfloat16
    wqT1 = np.ascontiguousarray(sa_wq.T, dtype=F32)
    wkT1 = np.ascontiguousarray(sa_wk.T.astype(BF16))
    wvT1 = np.ascontiguousarray(sa_wv.T.astype(BF16))
    wqT2 = np.ascontiguousarray(wq.T, dtype=F32)
    wkT2 = np.ascontiguousarray(wk.T.astype(BF16))
    wvT2 = np.ascontiguousarray(wv.T.astype(BF16))

    consts = np.zeros((P, 10), dtype=F32)
    consts[:, 0] = sa_gamma[0]
    consts[:, 1] = gamma[0]
    sgb = (sa_gamma[0] * sa_bv).astype(F32)
    consts[:, 2] = sgb[0:P]
    consts[:, 3] = sgb[P:C]
    consts[0:D, 6] = sa_bq
    consts[0:D, 7] = sa_bk
    consts[0:D, 8] = bq
    consts[0:D, 9] = bk

    in_maps = []
    for g in range(NCORES):
        b, r = g // RSH, g % RSH
        xb = np.ascontiguousarray(x[b].reshape(C, N))
        mb = np.ascontiguousarray(mask[b].reshape(1, N))
        in_maps.append({
            "xf": np.ascontiguousarray(xb.astype(BF16)),
            "xc": np.ascontiguousarray(xb[:, r * R : (r + 1) * R]),
            "mcrow": np.ascontiguousarray(mb[:, r * R : (r + 1) * R]),
            "wqT1": wqT1, "wkT1": wkT1, "wvT1": wvT1,
            "wqT2": wqT2, "wkT2": wkT2, "wvT2": wvT2,
            "consts": consts,
        })
    return in_maps


def kernel(**inputs):
    from concourse import bass_utils

    if "nc" not in _CACHE:
        _CACHE["nc"] = _build_bass()
    nc = _CACHE["nc"]

    in_maps = _prep_inputs(**inputs)
    res = bass_utils.run_bass_kernel_spmd(
        nc, in_maps, core_ids=list(range(NCORES))
    )
    _CACHE["last_results"] = res

    out = np.empty((B, C, N), dtype=F32)
    for g in range(NCORES):
        b, r = g // RSH, g % RSH
        out[b, :, r * R : (r + 1) * R] = res.results[g]["outc"]
    return out.reshape(B, C, HH, WW)


# revision 21
# speedup vs baseline: 1.3479x; 1.1268x over previous
"""Trainium2 Bass/Tile kernel for nn_FB_FMM (sparse_attention).

Computation (per batch element b, with N = H*W = 4096 tokens, C=256, D=32):
  1. Self-attention:  sa_out = attn(conv(x,sa_wq), conv(x,sa_wk), conv(x,sa_wv))
     x' = sa_gamma * sa_out + x
  2. Masked cross-attention (FB_FMM):
     ff = mask * x'; fb = (1-mask) * x'
     sw_bg = attn(conv(ff,wq), conv(fb,wk), conv(fb,wv))
     out = x' + gamma * ff * (std(sw_bg)/std(ff))    [per-channel std, ddof=1]

Sharding: 8 cores = 2 batch groups x 4-way query-row sharding (1024 rows each).
Each core computes its row-chunk of both attention layers; K/V sides are
computed redundantly per core. Cross-core communication inside the kernel:
  - AllGather of fb = (1-mask)x' chunks within each 4-core batch group
    (layer-2 K/V convs need full fb; gathering fb instead of x' removes all
    post-gather mask arithmetic), split into two 512-row phases so phase 0
    overlaps the second layer-1 attention i-chunk.
  - AllGather (+ local 3-add sum) of per-channel [sum, sumsq] stats for the
    FMM std ratio -- cheaper than a CC AllReduce for 4KB.

Layouts: feature maps are channel-major (C on partitions). Scores are computed
transposed (S^T: keys j on partitions, queries i free; logits are small so exp
needs no max-subtraction pass). The AV matmul keeps V^T slices stationary
with E^T moving, producing O in natural (c x i) layout; the softmax
denominator comes from one extra M=1 ones-matmul per tile, and the
reciprocal row (reciprocal_approx_fast, ~18 bits) is broadcast across
partitions with a K=1 ones matmul. Layer-1 K/V convs are emitted interleaved
with the i-chunk-0 attention tiles so the PE starts as soon as the first
input chunk lands instead of after the full 2MB x DMA.
"""

import numpy as np

P = 128
B, C, HH, WW = 2, 256, 64, 64
N = HH * WW            # 4096 tokens
D = 32                 # q/k channels
NCORES = 8
RSH = 4                # row shards per batch group
R = N // RSH           # 1024 query rows per core
NT = N // P            # 32 key tiles
IC = 512               # query i-chunk (one PSUM bank of fp32)
EPS = 1e-5
F32 = np.float32

_CACHE = {}


def _build_bass():
    """Build the Bass/Tile program (single SPMD NEFF for all 8 cores)."""
    import concourse.bass as bass
    from concourse import bacc, mybir, tile

    f32 = mybir.dt.float32
    f32r = mybir.dt.float32r
    bf16 = mybir.dt.bfloat16
    AX = mybir.AxisListType
    OP = mybir.AluOpType
    AF = mybir.ActivationFunctionType

    nc = bacc.Bacc(
        "TRN2", target_bir_lowering=False, debug=False, num_devices=NCORES
    )
    bf16d = mybir.dt.bfloat16

    # ---------------- I/O ----------------
    xf_d = nc.dram_tensor("xf", [C, N], bf16d, kind="ExternalInput")
    xc_d = nc.dram_tensor("xc", [C, R], f32r, kind="ExternalInput")
    mcrow_d = nc.dram_tensor("mcrow", [1, R], f32, kind="ExternalInput")
    # packed weights: one f32r pack (wq of both layers), one bf16 pack
    # (wk of both layers + wv of both layers) -- 2 DMAs instead of 12
    wpr_d = nc.dram_tensor("wpackr", [C, 2, D], f32r, kind="ExternalInput")
    wpb_d = nc.dram_tensor(
        "wpackb", [C, 2 * D + 2 * C], bf16d, kind="ExternalInput"
    )
    # consts columns: 0 sa_gamma, 1 gamma, 2/3 sa_gamma*sa_bv halves,
    # 6 sa_bq, 7 sa_bk, 8 bq, 9 bk (cols 6-9 live on partitions 0..31)
    consts_d = nc.dram_tensor("consts", [P, 10], f32, kind="ExternalInput")
    out_d = nc.dram_tensor("outc", [C, R], f32, kind="ExternalOutput")

    groups = [[0, 1, 2, 3], [4, 5, 6, 7]]

    with tile.TileContext(nc) as tc:
        from contextlib import ExitStack

        ctx = ExitStack()
        with ctx:
            big = ctx.enter_context(tc.tile_pool(name="big", bufs=1))
            epool = ctx.enter_context(tc.tile_pool(name="epool", bufs=7))
            dspool = ctx.enter_context(tc.tile_pool(name="dspool", bufs=3))
            onpool = ctx.enter_context(tc.tile_pool(name="onpool", bufs=3))
            sqpool = ctx.enter_context(tc.tile_pool(name="sqpool", bufs=2))
            rcpool = ctx.enter_context(tc.tile_pool(name="rcpool", bufs=2))
            finpool = ctx.enter_context(tc.tile_pool(name="finpool", bufs=2))
            misc = ctx.enter_context(tc.tile_pool(name="misc", bufs=1))
            # PSUM: accs (3 rotating) + S^T (2) + convs (2) + den/rrep (1)
            psO = ctx.enter_context(
                tc.tile_pool(name="psO", bufs=3, space="PSUM")
            )
            psS = ctx.enter_context(
                tc.tile_pool(name="psS", bufs=2, space="PSUM")
            )
            psC = ctx.enter_context(
                tc.tile_pool(name="psC", bufs=2, space="PSUM")
            )
            psD = ctx.enter_context(
                tc.tile_pool(name="psD", bufs=1, space="PSUM")
            )
            dram = ctx.enter_context(
                tc.tile_pool(name="dram", bufs=1, space="DRAM")
            )

            # ------------- persistent SBUF tiles -------------
            xc_sb = big.tile([P, 2, R], f32r, tag="xc", name="xc_sb")
            maskc_sb = big.tile([P, R], f32, tag="maskc", name="maskc_sb")
            xp_sb = big.tile([P, 2, R], f32, tag="xp", name="xp_sb")
            ff_sb = big.tile([P, 2, R], f32r, tag="ff", name="ff_sb")
            fb16_sb = big.tile([P, 2, R], bf16, tag="fb16", name="fb16_sb")
            wpr_sb = big.tile([P, 2, 2, D], f32r, tag="wpr", name="wpr_sb")
            wpb_sb = big.tile(
                [P, 2, 2 * D + 2 * C], bf16, tag="wpb", name="wpb_sb"
            )
            consts_sb = big.tile([P, 10], f32, tag="consts", name="consts_sb")
            # ones column (bf16) for the denominator matmul; ones row (f32r)
            # for the K=1 reciprocal-replication matmul
            onesc_sb = big.tile([P, 1], bf16, tag="onesc", name="onesc_sb")
            onesr_sb = big.tile([1, P], f32r, tag="onesr", name="onesr_sb")
            stats_sb = misc.tile([P, 8], f32, tag="stats", name="stats_sb")
            # per-(ct, ich) stat accumulator columns: 4 tensors x [ct, ich]
            sacc_sb = misc.tile([P, 16], f32, tag="sacc", name="sacc_sb")

            # input DMAs: dispatch is ~0.6us per dma_start on one engine's
            # queue, so spread across 4 engine queues and merge the C-half
            # pairs into single transposed-AP transfers.
            nc.sync.dma_start(out=consts_sb[:], in_=consts_d[:])
            nc.sync.dma_start(
                out=xc_sb[:],
                in_=xc_d[:].rearrange("(k p) n -> p k n", k=2),
            )
            nc.gpsimd.dma_start(
                out=wpr_sb[:],
                in_=wpr_d[:].rearrange("(k p) l d -> p k l d", k=2),
            )
            nc.gpsimd.dma_start(
                out=wpb_sb[:],
                in_=wpb_d[:].rearrange("(k p) x -> p k x", k=2),
            )
            nc.sync.dma_start(
                out=maskc_sb[:], in_=mcrow_d[0, :].partition_broadcast(P)
            )
            xf_sb = big.tile([P, 2, N], bf16, tag="xbig", name="xf_sb")
            NCH = 4                       # xf DMA chunks
            CW = N // NCH                 # 1024 columns per chunk
            for jc in range(NCH):
                js = slice(jc * CW, (jc + 1) * CW)
                eng = (nc.scalar, nc.gpsimd)[jc % 2]
                eng.dma_start(
                    out=xf_sb[:, :, js],
                    in_=xf_d[:, js].rearrange("(k p) n -> p k n", k=2),
                )
            nc.vector.memset(onesc_sb[:], 1.0)
            nc.vector.memset(onesr_sb[:].bitcast(f32), 1.0)

            def conv_qk(wT_sb, bias_col, src_of, cols, out_sb):
                """out[:, cols] (D x 512-chunks) = wT.T @ src + bias."""
                for jc in range(cols.start // IC, cols.stop // IC):
                    js = slice(jc * IC, (jc + 1) * IC)
                    ps = psC.tile([D, IC], f32, tag="c", name="qk_ps")
                    nc.tensor.matmul(
                        ps[:], wT_sb[:, 0, :], src_of(0, js),
                        start=True, stop=False,
                    )
                    nc.tensor.matmul(
                        ps[:], wT_sb[:, 1, :], src_of(1, js),
                        start=False, stop=True,
                    )
                    nc.vector.tensor_scalar_add(
                        out_sb[:, js], ps[:],
                        consts_sb[0:D, bias_col : bias_col + 1],
                    )

            def conv_vT(wvT_sb, src_of, v_sb, t):
                """v_sb[:, t, :] = (src^T @ wvT) for key tile t (j on
                partitions, channels free)."""
                ts_ = slice(t * P, (t + 1) * P)
                ps = psC.tile([P, C], f32, tag="c", name="v_ps")
                nc.tensor.matmul(
                    ps[:], src_of(0, ts_), wvT_sb[:, 0, :],
                    start=True, stop=False,
                )
                nc.tensor.matmul(
                    ps[:], src_of(1, ts_), wvT_sb[:, 1, :],
                    start=False, stop=True,
                )
                nc.vector.tensor_copy(v_sb[:, t, :], ps[:])

            class AttnChunk:
                """One query i-chunk of row-sharded attention, with tile
                emission split into arbitrary sub-sequences so conv work can
                be interleaved.  S^T = K-tile^T Q (j on partitions),
                E = exp(S^T), O accumulates with V^T stationary and E moving;
                denominator via an M=1 ones-matmul per tile."""

                GD = 4  # tiles per denominator group

                def __init__(self, q_sb, k_sb, v_sb, ich, order):
                    self.q_sb, self.k_sb, self.v_sb = q_sb, k_sb, v_sb
                    self.is_ = slice(ich * IC, (ich + 1) * IC)
                    self.order = order
                    self.pos = 0          # next order index to AV
                    self.ahead = 0        # next order index to S/exp
                    self.es = {}
                    self.group = []       # e-tiles awaiting the den group
                    self.gidx = 0
                    self.accs = [
                        psO.tile([P, IC], f32, tag="o", name="acc")
                        for _ in range(2)
                    ]
                    self.den = psD.tile([1, IC], f32, tag="d", name="den")

                def _s_exp(self):
                    t = self.order[self.ahead]
                    self.ahead += 1
                    sps = psS.tile([P, IC], f32, tag="s", name="s_ps")
                    nc.tensor.matmul(
                        sps[:],
                        self.k_sb[:, t * P : (t + 1) * P],
                        self.q_sb[:, self.is_],
                        start=True, stop=True,
                    )
                    e_sb = epool.tile([P, IC], bf16, tag="e", name="e_sb")
                    nc.scalar.activation(e_sb[:], sps[:], AF.Exp)
                    self.es[t] = e_sb

                def emit(self, n):
                    """Emit the next n tiles' worth of S/exp/AV; every GD
                    tiles the e-tiles are tree-summed on the DVE (bf16, 2x
                    mode) and a single ones-matmul accumulates the softmax
                    denominator -- 1 PE matmul + ldweights per GD tiles
                    instead of per tile."""
                    LOOKAHEAD = 1
                    for _ in range(n):
                        while (
                            self.ahead < len(self.order)
                            and self.ahead <= self.pos + LOOKAHEAD
                        ):
                            self._s_exp()
                        t = self.order[self.pos]
                        first = self.pos == 0
                        last = self.pos == len(self.order) - 1
                        self.pos += 1
                        e_sb = self.es.pop(t)
                        for ct in range(2):
                            nc.tensor.matmul(
                                self.accs[ct][:],
                                self.v_sb[:, t, ct * P : (ct + 1) * P],
                                e_sb[:],
                                start=first, stop=last,
                            )
                        self.group.append(e_sb)
                        if len(self.group) == self.GD:
                            g0, g1, g2, g3 = self.group
                            self.group = []
                            e01 = dspool.tile([P, IC], bf16, tag="ds",
                                              name="e01")
                            nc.vector.tensor_add(e01[:], g0[:], g1[:])
                            e23 = dspool.tile([P, IC], bf16, tag="ds",
                                              name="e23")
                            nc.vector.tensor_add(e23[:], g2[:], g3[:])
                            esum = dspool.tile([P, IC], bf16, tag="ds",
                                               name="esum")
                            nc.vector.tensor_add(esum[:], e01[:], e23[:])
                            ng = len(self.order) // self.GD
                            nc.tensor.matmul(
                                self.den[:], onesc_sb[:], esum[:],
                                start=self.gidx == 0,
                                stop=self.gidx == ng - 1,
                            )
                            self.gidx += 1

                def rrep(self):
                    """Reciprocal of the denominator row, replicated to all
                    partitions via a K=1 ones matmul (f32r single-pass).
                    reciprocal_approx_fast (~18 bits) replaces the 5x slower
                    exact DVE reciprocal; the f32r copy rounds for the PE."""
                    rrow = rcpool.tile([1, IC], f32, tag="rc0", name="rw")
                    nc.vector.reciprocal_approx_fast(
                        out=rrow[:], in_=self.den[:]
                    )
                    rrow_r = rcpool.tile([1, IC], f32r, tag="rc", name="rr")
                    nc.vector.tensor_copy(rrow_r[:], rrow[:])
                    rrep_ps = psD.tile([P, IC], f32, tag="d", name="rrep_ps")
                    nc.tensor.matmul(
                        rrep_ps[:], onesr_sb[:], rrow_r[:],
                        start=True, stop=True,
                    )
                    rr = onpool.tile([P, IC], f32, tag="rr", name="rrep")
                    nc.scalar.copy(rr[:], rrep_ps[:])
                    return rr

            # ================= Layer 1: self-attention =================
            q1_sb = big.tile([D, R], bf16, tag="q", name="q1_sb")
            k1_sb = big.tile([D, N], bf16, tag="k", name="k1_sb")
            v1_sb = big.tile([P, NT, C], bf16, tag="v", name="v1_sb")

            wq1 = wpr_sb[:, :, 0, :]
            wq2 = wpr_sb[:, :, 1, :]
            wk1 = wpb_sb[:, :, 0:D]
            wk2 = wpb_sb[:, :, D : 2 * D]
            wv1 = wpb_sb[:, :, 2 * D : 2 * D + C]
            wv2 = wpb_sb[:, :, 2 * D + C : 2 * D + 2 * C]

            conv_qk(wq1, 6, lambda k, js: xc_sb[:, k, js], slice(0, R),
                    q1_sb)

            def l1_conv_chunk(c):
                js = slice(c * CW, (c + 1) * CW)
                conv_qk(wk1, 7, lambda k, js_: xf_sb[:, k, js_], js, k1_sb)
                for t in range(8 * c, 8 * c + 8):
                    conv_vT(wv1, lambda k, ts_: xf_sb[:, k, ts_], v1_sb, t)

            # i-chunk 0 attention trails the conv chunks by one chunk
            at0 = AttnChunk(q1_sb, k1_sb, v1_sb, 0, list(range(NT)))
            l1_conv_chunk(0)
            for c in range(1, NCH):
                l1_conv_chunk(c)
                at0.emit(8)
            at0.emit(8)

            ag_ins, ag_outs = [], []
            for h in range(2):
                ag_ins.append(dram.tile(
                    [C, IC], bf16, tag=f"ag_in{h}", name=f"ag_in{h}"
                ))
                ag_outs.append(dram.tile(
                    [RSH, C, IC], bf16, tag=f"ag_out{h}", name=f"ag_out{h}"
                ))

            def epilogue1(ich, at):
                """x' = sa_gamma*(O/den) + sa_gamma*bv + x, then ff = m*x'
                (f32r, with Σff accumulated), fb16 = x' - ff for the gather,
                and Σff² -- stats land in sacc_sb columns."""
                rr = at.rrep()
                io = slice(ich * IC, (ich + 1) * IC)
                for ct in range(2):
                    nc.vector.scalar_tensor_tensor(
                        xp_sb[:, ct, io], at.accs[ct][:],
                        consts_sb[:, 0:1], rr[:],
                        op0=OP.mult, op1=OP.mult,
                    )
                    nc.vector.scalar_tensor_tensor(
                        xp_sb[:, ct, io], xp_sb[:, ct, io],
                        consts_sb[:, 2 + ct : 3 + ct],
                        xc_sb[:, ct, io].bitcast(f32),
                        op0=OP.add, op1=OP.add,
                    )
                    nc.vector.scalar_tensor_tensor(
                        ff_sb[:, ct, io], xp_sb[:, ct, io],
                        1.0, maskc_sb[:, io],
                        op0=OP.mult, op1=OP.mult,
                        accum_out=sacc_sb[:, 2 * ct + ich : 2 * ct + ich + 1],
                    )
                    nc.vector.tensor_sub(
                        fb16_sb[:, ct, io], xp_sb[:, ct, io],
                        ff_sb[:, ct, io].bitcast(f32),
                    )
                # AllGather this i-chunk's fb columns ASAP
                nc.sync.dma_start(
                    out=ag_ins[ich][:].rearrange("(ct p) n -> p ct n", ct=2),
                    in_=fb16_sb[:, :, io],
                )
                nc.gpsimd.collective_compute(
                    "AllGather",
                    OP.bypass,
                    replica_groups=groups,
                    ins=[ag_ins[ich][:].opt()],
                    outs=[ag_outs[ich][:].opt()],
                )
                # Σff² (scratch output, accumulator is the point)
                for ct in range(2):
                    sq = sqpool.tile([P, IC], f32, tag="sq", name="ffsq")
                    nc.vector.scalar_tensor_tensor(
                        sq[:], ff_sb[:, ct, io].bitcast(f32),
                        1.0, ff_sb[:, ct, io].bitcast(f32),
                        op0=OP.mult, op1=OP.mult,
                        accum_out=sacc_sb[:, 4 + 2 * ct + ich
                                          : 5 + 2 * ct + ich],
                    )

            epilogue1(0, at0)
            del at0

            at1 = AttnChunk(q1_sb, k1_sb, v1_sb, 1, list(range(NT)))
            at1.emit(NT)
            epilogue1(1, at1)
            del at1

            # assemble ff stats: stats cols 0-1 = S1f (ct0, ct1),
            # cols 4-5 = S2f -- S1/S2 grouped 4-wide for the var chain
            for c in range(4):
                dst = (0, 1, 4, 5)[c]
                nc.vector.tensor_add(
                    stats_sb[:, dst : dst + 1],
                    sacc_sb[:, 2 * c : 2 * c + 1],
                    sacc_sb[:, 2 * c + 1 : 2 * c + 2],
                )

            # ============== Layer 2: masked cross-attention ==============
            q2_sb = big.tile([D, R], bf16, tag="q", name="q2_sb")
            conv_qk(wq2, 8, lambda k, js: ff_sb[:, k, js], slice(0, R),
                    q2_sb)

            # gathered fb landing buffer; phase h delivers columns
            # [r*R + h*IC, r*R + (h+1)*IC) for every rank r
            fbf_sb = big.tile([P, 2, N], bf16, tag="xbig", name="fbf_sb")
            for h in range(2):
                for r in range(RSH):
                    eng = (nc.sync, nc.gpsimd)[r % 2]
                    eng.dma_start(
                        out=fbf_sb[
                            :, :, r * R + h * IC : r * R + (h + 1) * IC
                        ],
                        in_=ag_outs[h][r].rearrange(
                            "(ct p) n -> p ct n", ct=2
                        ),
                    )

            k2_sb = big.tile([D, N], bf16, tag="k", name="k2_sb")
            v2_sb = big.tile([P, NT, C], bf16, tag="v", name="v2_sb")

            def l2_conv_block(r, h):
                js = slice(r * R + h * IC, r * R + (h + 1) * IC)
                conv_qk(wk2, 9, lambda k, js_: fbf_sb[:, k, js_], js, k2_sb)
                t0 = (r * R + h * IC) // P
                for t in range(t0, t0 + 4):
                    conv_vT(wv2, lambda k, ts_: fbf_sb[:, k, ts_], v2_sb, t)

            ph_tiles = [
                [t for r in range(RSH)
                 for t in range((r * R + h * IC) // P,
                                (r * R + h * IC) // P + 4)]
                for h in range(2)
            ]

            for r in range(RSH):
                l2_conv_block(r, 0)
            bt0 = AttnChunk(q2_sb, k2_sb, v2_sb, 0,
                            ph_tiles[0] + ph_tiles[1])
            bt0.emit(10)
            for r in range(RSH):
                l2_conv_block(r, 1)
            bt0.emit(NT - 10)

            def epilogue2(ich, at):
                """normalized sw_bg chunk; accumulate per-channel sum/sumsq
                into sacc_sb cols 8-15 via the DVE accumulator."""
                rr = at.rrep()
                for ct in range(2):
                    onb = onpool.tile([P, IC], f32, tag="rr", name="on2")
                    nc.vector.scalar_tensor_tensor(
                        onb[:], at.accs[ct][:], 1.0, rr[:],
                        op0=OP.mult, op1=OP.mult,
                        accum_out=sacc_sb[:, 8 + 2 * ct + ich
                                          : 9 + 2 * ct + ich],
                    )
                    sqb = sqpool.tile([P, IC], f32, tag="sq", name="sq2")
                    nc.vector.scalar_tensor_tensor(
                        sqb[:], onb[:], 1.0, onb[:],
                        op0=OP.mult, op1=OP.mult,
                        accum_out=sacc_sb[:, 12 + 2 * ct + ich
                                          : 13 + 2 * ct + ich],
                    )

            epilogue2(0, bt0)
            del bt0
            bt1 = AttnChunk(q2_sb, k2_sb, v2_sb, 1, list(range(NT)))
            bt1.emit(NT)
            epilogue2(1, bt1)
            del bt1

            # assemble sw_bg stats: stats cols 2-3 = S1g, cols 6-7 = S2g
            for c in range(4):
                dst = (2, 3, 6, 7)[c]
                nc.vector.tensor_add(
                    stats_sb[:, dst : dst + 1],
                    sacc_sb[:, 8 + 2 * c : 9 + 2 * c],
                    sacc_sb[:, 9 + 2 * c : 10 + 2 * c],
                )

            # ============ stats AllGather + local sum + FMM ============
            sg_in = dram.tile([P, 8], f32, tag="sg_in", name="sg_in")
            sg_out = dram.tile([RSH, P, 8], f32, tag="sg_out", name="sg_out")
            nc.sync.dma_start(out=sg_in[:], in_=stats_sb[:])
            nc.gpsimd.collective_compute(
                "AllGather",
                OP.bypass,
                replica_groups=groups,
                ins=[sg_in[:].opt()],
                outs=[sg_out[:].opt()],
            )
            rst4 = misc.tile([P, RSH, 8], f32, tag="rst4", name="rst4")
            nc.sync.dma_start(
                out=rst4[:], in_=sg_out[:].transpose([1, 0, 2])
            )
            rst = misc.tile([P, 8], f32, tag="rst", name="rst")
            nc.vector.tensor_add(rst[:], rst4[:, 0, :], rst4[:, 1, :])
            rstb = misc.tile([P, 8], f32, tag="rstb", name="rstb")
            nc.vector.tensor_add(rstb[:], rst4[:, 2, :], rst4[:, 3, :])
            nc.vector.tensor_add(rst[:], rst[:], rstb[:])

            # var = (S2 - S1^2/N)/(N-1) + EPS, all four (f/g x ct) at once
            var4 = misc.tile([P, 4], f32, tag="var4", name="var4")
            ratio = misc.tile([P, 2], f32, tag="ratio", name="ratio")
            nc.vector.tensor_mul(var4[:], rst[:, 0:4], rst[:, 0:4])
            nc.vector.tensor_scalar(
                var4[:], var4[:], -1.0 / N, None, op0=OP.mult
            )
            nc.vector.tensor_add(var4[:], var4[:], rst[:, 4:8])
            nc.vector.tensor_scalar(
                var4[:], var4[:], 1.0 / (N - 1), EPS, op0=OP.mult, op1=OP.add
            )
            varfi = misc.tile([P, 2], f32, tag="varfi", name="varfi")
            nc.vector.reciprocal(varfi[:], var4[:, 0:2])
            nc.vector.tensor_mul(varfi[:], varfi[:], var4[:, 2:4])
            nc.scalar.activation(ratio[:], varfi[:], AF.Sqrt)
            # fold in gamma
            nc.vector.tensor_scalar_mul(ratio[:], ratio[:], consts_sb[:, 1:2])

            # out = x' + (gamma * std_bg/std_f) * ff, in 4 chunks with the
            # store DMAs dispatched on 4 different engine queues
            outq = (nc.sync, nc.scalar, nc.gpsimd, nc.sync)
            for i, (ct, ih) in enumerate(
                ((0, 0), (1, 0), (0, 1), (1, 1))
            ):
                io = slice(ih * IC, (ih + 1) * IC)
                fin = finpool.tile([P, IC], f32, tag="fin", name="fin")
                nc.vector.scalar_tensor_tensor(
                    fin[:], ff_sb[:, ct, io].bitcast(f32),
                    ratio[:, ct : ct + 1], xp_sb[:, ct, io],
                    op0=OP.mult, op1=OP.add,
                )
                outq[i].dma_start(
                    out=out_d[ct * P : (ct + 1) * P, io], in_=fin[:]
                )

    nc.compile()
    return nc


def _prep_inputs(x, mask, sa_wq, sa_bq, sa_wk, sa_bk, sa_wv, sa_bv, sa_gamma,
                 wq, bq, wk, bk, wv, bv, gamma):
    """Build the per-core input maps (host-side sharding + weight layout)."""
    x = np.ascontiguousarray(x, dtype=F32)
    mask = np.ascontiguousarray(mask, dtype=F32)

    import ml_dtypes

    BF16 = ml_dtypes.bfloat16
    # packed weights: f32r pack [C, 2, D] = (wq layer1, wq layer2);
    # bf16 pack [C, 2D+2C] = wk1 | wk2 | wv1 | wv2
    wpackr = np.ascontiguousarray(
        np.stack([sa_wq.T, wq.T], axis=1), dtype=F32
    )
    wpackb = np.ascontiguousarray(
        np.concatenate([sa_wk.T, wk.T, sa_wv.T, wv.T], axis=1).astype(BF16)
    )

    consts = np.zeros((P, 10), dtype=F32)
    consts[:, 0] = sa_gamma[0]
    consts[:, 1] = gamma[0]
    sgb = (sa_gamma[0] * sa_bv).astype(F32)
    consts[:, 2] = sgb[0:P]
    consts[:, 3] = sgb[P:C]
    consts[0:D, 6] = sa_bq
    consts[0:D, 7] = sa_bk
    consts[0:D, 8] = bq
    consts[0:D, 9] = bk

    in_maps = []
    for g in range(NCORES):
        b, r = g // RSH, g % RSH
        xb = np.ascontiguousarray(x[b].reshape(C, N))
        mb = np.ascontiguousarray(mask[b].reshape(1, N))
        in_maps.append({
            "xf": np.ascontiguousarray(xb.astype(BF16)),
            "xc": np.ascontiguousarray(xb[:, r * R : (r + 1) * R]),
            "mcrow": np.ascontiguousarray(mb[:, r * R : (r + 1) * R]),
            "wpackr": wpackr, "wpackb": wpackb,
            "consts": consts,
        })
    return in_maps


def kernel(**inputs):
    from concourse import bass_utils

    if "nc" not in _CACHE:
        _CACHE["nc"] = _build_bass()
    nc = _CACHE["nc"]

    in_maps = _prep_inputs(**inputs)
    res = bass_utils.run_bass_kernel_spmd(
        nc, in_maps, core_ids=list(range(NCORES))
    )
    _CACHE["last_results"] = res

    out = np.empty((B, C, N), dtype=F32)
    for g in range(NCORES):
        b, r = g // RSH, g % RSH
        out[b, :, r * R : (r + 1) * R] = res.results[g]["outc"]
    return out.reshape(B, C, HH, WW)
